# revision 1
# baseline (speedup 1.0000x reference)
"""TRN2 Bass kernel for the vq_codebook problem (nn_DNN_34497177321482).

kernel(**inputs) -> np.ndarray  [full-shape in, full-shape out]

Strategy (8 NeuronCores, data-parallel over batch; 64 batches/core):
  - Host packs the active (mask>=1) history positions per core into tiles of
    128 rows; embedding_table (pre-scaled x8) is gathered on-device via
    indirect DMA.
  - Distances: sadj[r,n] = G*(||c_n||^2/2 - x_r.c_n) (same argmin as full
    squared distance), G=512.  The GEMM is one fp16 pass xh.(ch*G) plus two
    fp8-e4m3 DoubleRow (double-pumped) correction passes xh8.(cl*G)8 and
    (xl*G)8.ch8 -- the scale split keeps every fp8 operand in e4m3's dynamic
    range while all three products land at the same PSUM scale.
    Row-min via DVE reduce; one-hot eqm1 = Sign(min - sadj) in {-1,0}.
  - Per-batch reductions as PE matmuls against a 0/1 membership matrix S:
    cnt' = S^T eqm1 (counts minus batch-size) and hist = S^T xh (single fp16
    term).  Masked rows quantize to the min-norm code n0: host adds
    (L-cnt_b) at column n0; the constant colsum(cb)@W1 term is folded into
    the bias.
  - stage2: cntT @ codebook (fp16 pair) -> vq_sum; then
    [vq_mean, hist_mean] @ W_enc (fp16) + bias on-device; gathered on host.
"""

import sys

sys.path.insert(0, "/opt/trn_rl_repo")

import numpy as np
import ml_dtypes

import concourse.bacc as bacc
import concourse.bass as bass
import concourse.tile as tile
import concourse.mybir as mybir
from concourse.bass_utils import run_bass_kernel_spmd
from concourse.masks import make_identity

F32 = mybir.dt.float32
F16 = mybir.dt.float16
F8 = mybir.dt.float8e4
I32 = mybir.dt.int32
NP8 = ml_dtypes.float8_e4m3

V, D, K, L, B = 100000, 256, 2048, 200, 512
N_CORES = 8
BL = B // N_CORES
KC = D // 128          # main-GEMM contraction chunks
NC = K // 512          # distance n-chunks
SC = K // 128          # stage2 contraction chunks
XC = (2 * D) // 128    # final dense contraction chunks
SCALE = 8.0
G = 512.0

_program_cache = {}


def _build_program(T):
    nc = bacc.Bacc("TRN2", target_bir_lowering=False, debug=False,
                   enable_asserts=False, num_devices=N_CORES)

    def din(name, shape, dt):
        return nc.dram_tensor(name, shape, dt, kind="ExternalInput").ap()

    emb_d = din("emb", [V, D], F32)
    ids_d = din("ids", [128, T], I32)
    s16_d = din("s16", [128, T * BL], F16)
    normsb_d = din("normsb", [128, K], F32)
    chT9_d = din("chT9", [128, KC * K], F16)
    cl9T8_d = din("cl9T8", [128, KC * K], F8)
    chT8_d = din("chT8", [128, KC * K], F8)
    ch2_d = din("ch2", [128, SC * D], F16)
    cl2_d = din("cl2", [128, SC * D], F16)
    cbc_d = din("cbc", [BL, D], F32)
    rvq_d = din("rvq", [BL, 1], F32)
    rhist_d = din("rhist", [BL, 1], F32)
    corr_d = din("corr", [BL, 1], F32)
    g01_d = din("g01", [128, 2 * D], F32)
    wT_d = din("wT", [128, XC * D], F16)
    bias_d = din("bias", [BL, D], F32)
    out_d = nc.dram_tensor("out", [BL, D], F32, kind="ExternalOutput").ap()

    def emit(tc, n0):
        const = tc.alloc_tile_pool(name="const", bufs=1)
        ppersist = tc.alloc_tile_pool(name="ppersist", bufs=1, space="PSUM")

        ident = const.tile([128, 128], F32, name="ident")
        make_identity(nc, ident[:])

        ids_sb = const.tile([128, T], I32, name="ids_sb")
        nc.gpsimd.dma_start(ids_sb[:], ids_d)
        p_gx = tc.alloc_tile_pool(name="gx", bufs=6)
        pregath = {}
        for t in range(min(2, T)):
            gx = p_gx.tile([128, D], F32, tag="gx", name=f"gx{t}")
            nc.sync.dma_start(gx[:], g01_d[:, t * D:(t + 1) * D])
            pregath[t] = gx
        chT9_sb = const.tile([128, KC * K], F16, name="chT9_sb")
        for kc in range(KC):
            nc.sync.dma_start(chT9_sb[:, kc * K:(kc + 1) * K],
                              chT9_d[:, kc * K:(kc + 1) * K])
        cl9T8_sb = const.tile([128, KC, K], F8, name="cl9T8_sb")
        for kc in range(KC):
            nc.sync.dma_start(cl9T8_sb[:, kc:kc + 1, :],
                              cl9T8_d[:, kc * K:(kc + 1) * K])
        chT8_sb = const.tile([128, KC, K], F8, name="chT8_sb")
        for kc in range(KC):
            nc.sync.dma_start(chT8_sb[:, kc:kc + 1, :],
                              chT8_d[:, kc * K:(kc + 1) * K])
        normsb_sb = const.tile([128, K], F32, name="normsb_sb")
        nc.sync.dma_start(normsb_sb[:], normsb_d)
        s16_sb = const.tile([128, T * BL], F16, name="s16_sb")
        nc.sync.dma_start(s16_sb[:], s16_d)
        # late consts: tiles now, DMAs after the tile loop
        ch2_sb = const.tile([128, SC * D], F16, name="ch2_sb")
        cl2_sb = const.tile([128, SC * D], F16, name="cl2_sb")
        cbc_sb = const.tile([BL, D], F32, name="cbc_sb")
        rvq_sb = const.tile([BL, 1], F32, name="rvq_sb")
        rhist_sb = const.tile([BL, 1], F32, name="rhist_sb")
        corr_sb = const.tile([BL, 1], F32, name="corr_sb")
        wT_sb = const.tile([128, XC * D], F16, name="wT_sb")
        bias_sb = const.tile([BL, D], F32, name="bias_sb")

        cnt_ps = ppersist.tile([128, K // 2], F32, space="PSUM", name="cnt_ps")
        hist_ps = ppersist.tile([BL, D], F32, space="PSUM", name="hist_ps")

        p_xtps = tc.alloc_tile_pool(name="xtps", bufs=2, space="PSUM")
        p_xt = tc.alloc_tile_pool(name="xt", bufs=3)
        p_gh = tc.alloc_tile_pool(name="gh", bufs=3)
        p_sps = tc.alloc_tile_pool(name="sps", bufs=3, space="PSUM")
        p_sadj = tc.alloc_tile_pool(name="sadj", bufs=3)
        p_eq = tc.alloc_tile_pool(name="eq", bufs=3)
        p_m = tc.alloc_tile_pool(name="m", bufs=3)

        for t in range(T):
            if t in pregath:
                gx = pregath[t]
            else:
                gx = p_gx.tile([128, D], F32, tag="gx", name=f"gx{t}")
                nc.gpsimd.indirect_dma_start(
                    out=gx[:], out_offset=None, in_=emb_d,
                    in_offset=bass.IndirectOffsetOnAxis(ap=ids_sb[:, t:t + 1],
                                                        axis=0),
                )
            xt_ps = p_xtps.tile([128, D], F32, tag="xtps", space="PSUM",
                                name=f"xtps{t}")
            for kc in range(KC):
                nc.tensor.transpose(xt_ps[:, kc * 128:(kc + 1) * 128],
                                    gx[:, kc * 128:(kc + 1) * 128], ident[:])
            xh = p_xt.tile([128, D], F16, tag="xh", name=f"xh{t}")
            nc.scalar.copy(xh[:], xt_ps[:])
            xl16 = p_xt.tile([128, D], F16, tag="xl16", name=f"xl16{t}")
            nc.vector.tensor_tensor(out=xl16[:], in0=xt_ps[:], in1=xh[:],
                                    op=mybir.AluOpType.subtract)
            xl8 = p_xt.tile([128, KC, 128], F8, tag="xl8", name=f"xl8{t}")
            nc.scalar.activation(xl8[:, :, :], xl16[:],
                                 mybir.ActivationFunctionType.Copy,
                                 scale=G)
            xh8 = p_xt.tile([128, KC, 128], F8, tag="xh8", name=f"xh8{t}")
            nc.gpsimd.tensor_copy(xh8[:, :, :], xh[:])
            gh = p_gh.tile([128, D], F16, tag="gh", name=f"gh{t}")
            nc.gpsimd.tensor_copy(gh[:], gx[:])

            sadj = p_sadj.tile([128, K], F32, tag="sadj", name=f"sadj{t}")
            m_parts = p_m.tile([128, max(NC, 8)], F32, tag="mparts",
                               name=f"mp{t}")
            for nch in range(NC):
                sl = slice(nch * 512, (nch + 1) * 512)
                s_ps = p_sps.tile([128, 512], F32, tag="sps", space="PSUM",
                                  name=f"sps{t}_{nch}")
                for kc in range(KC):
                    ksl = slice(kc * 128, (kc + 1) * 128)
                    csl = slice(kc * K + nch * 512, kc * K + (nch + 1) * 512)
                    nc.tensor.matmul(s_ps[:], xh[:, ksl], chT9_sb[:, csl],
                                     start=(kc == 0), stop=False)
                nc.tensor.matmul(s_ps[:], xh8[:, :, :], cl9T8_sb[:, :, sl],
                                 start=False, stop=False,
                                 perf_mode=mybir.MatmulPerfMode.DoubleRow)
                nc.tensor.matmul(s_ps[:], xl8[:, :, :], chT8_sb[:, :, sl],
                                 start=False, stop=True,
                                 perf_mode=mybir.MatmulPerfMode.DoubleRow)
                nc.vector.tensor_tensor(out=sadj[:, sl], in0=normsb_sb[:, sl],
                                        in1=s_ps[:], op=mybir.AluOpType.subtract)
                nc.vector.tensor_reduce(out=m_parts[:, nch:nch + 1],
                                        in_=sadj[:, sl],
                                        axis=mybir.AxisListType.X,
                                        op=mybir.AluOpType.min)
            m_min = p_m.tile([128, 1], F32, tag="m", name=f"m{t}")
            nc.vector.tensor_reduce(out=m_min[:], in_=m_parts[:, :NC],
                                    axis=mybir.AxisListType.X,
                                    op=mybir.AluOpType.min)

            eq = p_eq.tile([128, K], F16, tag="eq", name=f"eq{t}")
            nc.scalar.activation(eq[:], sadj[:],
                                 mybir.ActivationFunctionType.Sign,
                                 bias=m_min[:], scale=-1.0)

            ssl = slice(t * BL, (t + 1) * BL)
            for nch in range(NC):
                sl = slice(nch * 512, (nch + 1) * 512)
                po = 0 if nch < NC // 2 else 64
                psl = slice((nch % (NC // 2)) * 512, (nch % (NC // 2)) * 512 + 512)
                nc.tensor.matmul(cnt_ps[po:po + BL, psl], s16_sb[:, ssl],
                                 eq[:, sl], start=(t == 0), stop=(t == T - 1))
            nc.tensor.matmul(hist_ps[:], s16_sb[:, ssl], gh[:],
                             start=(t == 0), stop=(t == T - 1))

        nc.sync.dma_start(ch2_sb[:], ch2_d)
        nc.sync.dma_start(cl2_sb[:], cl2_d)
        nc.sync.dma_start(cbc_sb[:], cbc_d)
        nc.sync.dma_start(rvq_sb[:], rvq_d)
        nc.sync.dma_start(rhist_sb[:], rhist_d)
        nc.sync.dma_start(corr_sb[:], corr_d)
        nc.sync.dma_start(wT_sb[:], wT_d)
        nc.sync.dma_start(bias_sb[:], bias_d)

        for p in (p_m, p_eq, p_sadj, p_sps, p_gh, p_xt, p_xtps, p_gx):
            p.release()

        # ---- final phase ----
        fin = tc.alloc_tile_pool(name="fin", bufs=1)
        pfinA = tc.alloc_tile_pool(name="pfinA", bufs=1, space="PSUM")

        cntT = fin.tile([BL, K], F32, name="cntT")
        for nch in range(NC):
            sl = slice(nch * 512, (nch + 1) * 512)
            po = 0 if nch < NC // 2 else 64
            psl = slice((nch % (NC // 2)) * 512, (nch % (NC // 2)) * 512 + 512)
            nc.vector.tensor_copy(cntT[:, sl], cnt_ps[po:po + BL, psl])
            if nch * 512 <= n0 < (nch + 1) * 512:
                nc.vector.tensor_tensor(
                    out=cntT[:, n0:n0 + 1], in0=cntT[:, n0:n0 + 1],
                    in1=corr_sb[:], op=mybir.AluOpType.add)

        vq_ps = pfinA.tile([BL, D], F32, tag="vqps", space="PSUM", name="vqps")
        for kc in range(SC):
            ctp = pfinA.tile([128, BL], F32, tag="ctp", space="PSUM", bufs=2,
                             name=f"ctp{kc}")
            nc.tensor.transpose(ctp[:], cntT[:, kc * 128:(kc + 1) * 128],
                                ident[:BL, :BL])
            cc = fin.tile([128, BL], F16, tag=f"cc{kc % 2}", name=f"cc{kc}")
            nc.vector.tensor_copy(cc[:], ctp[:])
            dsl = slice(kc * D, (kc + 1) * D)
            nc.tensor.matmul(vq_ps[:], cc[:], ch2_sb[:, dsl],
                             start=(kc == 0), stop=False)
            nc.tensor.matmul(vq_ps[:], cc[:], cl2_sb[:, dsl],
                             start=False, stop=(kc == SC - 1))

        x_sb = fin.tile([BL, 2 * D], F32, name="x_sb")
        nc.vector.tensor_scalar_mul(x_sb[:, 0:D], vq_ps[:], rvq_sb[:])
        nc.vector.tensor_tensor(out=x_sb[:, 0:D], in0=x_sb[:, 0:D],
                                in1=cbc_sb[:], op=mybir.AluOpType.add)
        nc.vector.tensor_scalar_mul(x_sb[:, D:2 * D], hist_ps[:], rhist_sb[:])

        pfinA.release()
        pfinB = tc.alloc_tile_pool(name="pfinB", bufs=1, space="PSUM")
        out_ps = pfinB.tile([BL, D], F32, tag="outps", space="PSUM",
                            name="outps")
        for c in range(XC):
            xtp = pfinB.tile([128, BL], F32, tag="xtp", space="PSUM", bufs=2,
                             name=f"xtp{c}")
            nc.tensor.transpose(xtp[:], x_sb[:, c * 128:(c + 1) * 128],
                                ident[:BL, :BL])
            xc = fin.tile([128, BL], F16, tag=f"xc{c % 2}", name=f"xc{c}")
            nc.vector.tensor_copy(xc[:], xtp[:])
            nc.tensor.matmul(out_ps[:], xc[:], wT_sb[:, c * D:(c + 1) * D],
                             start=(c == 0), stop=(c == XC - 1))

        out_sb = fin.tile([BL, D], F32, name="out_sb")
        nc.vector.tensor_tensor(out=out_sb[:], in0=out_ps[:], in1=bias_sb[:],
                                op=mybir.AluOpType.add)
        nc.sync.dma_start(out_d, out_sb[:])

        for p in (pfinB, fin, ppersist, const):
            p.release()

    return nc, emit


def _get_program(T, n0):
    key = (T, n0)
    if key not in _program_cache:
        nc, emit = _build_program(T)
        with tile.TileContext(nc) as tc:
            emit(tc, n0)
        nc.compile()
        _program_cache[key] = nc
    return _program_cache[key]


def kernel(history_item_ids, history_item_masks, embedding_table, code_book,
           W_enc, b_enc):
    ids = np.asarray(history_item_ids)
    masks = np.asarray(history_item_masks)
    E = np.asarray(embedding_table, dtype=np.float32)
    C = np.asarray(code_book, dtype=np.float32)
    W = np.asarray(W_enc, dtype=np.float32)
    b = np.asarray(b_enc, dtype=np.float32)
    assert ids.shape == (B, L) and E.shape == (V, D) and C.shape == (K, D)

    mask = (masks >= 1)
    cnt = mask.sum(axis=1).astype(np.float64)                   # [B]
    n_act = [int(mask[c * BL:(c + 1) * BL].sum()) for c in range(N_CORES)]
    T = max(1, -(-max(n_act) // 128))

    norms = (C ** 2).sum(axis=1, dtype=np.float32)              # fp32 like ref
    n0 = int(np.argmin(norms))

    sc = np.float32(SCALE)
    g = np.float32(G)
    Emb_scaled = E * sc
    Cs = C * sc
    Ch = Cs.astype(np.float16)
    Clf = Cs - Ch.astype(np.float32)                            # fp16 residual

    normsb_row = (norms.astype(np.float64) * float(sc) ** 2 * float(g) / 2.0
                  ).astype(np.float32)
    normsb = np.broadcast_to(normsb_row, (128, K)).copy()

    colsum = C.sum(axis=0, dtype=np.float64)
    bias_bcast = np.broadcast_to(b.astype(np.float32), (BL, D)).copy()
    cbc_bcast = np.broadcast_to(colsum.astype(np.float32), (BL, D)).copy()

    wT = np.zeros((128, XC * D), np.float16)
    for c in range(XC):
        wT[:, c * D:(c + 1) * D] = W[c * 128:(c + 1) * 128]

    Ch9 = (Ch.astype(np.float32) * g).astype(np.float16)        # exact (pow2)
    chT9 = np.zeros((128, KC * K), np.float16)
    cl9T8 = np.zeros((128, KC * K), NP8)
    chT8 = np.zeros((128, KC * K), NP8)
    for kc in range(KC):
        chT9[:, kc * K:(kc + 1) * K] = Ch9[:, kc * 128:(kc + 1) * 128].T
        cl9T8[:, kc * K:(kc + 1) * K] = (
            Clf[:, kc * 128:(kc + 1) * 128].T * float(g)).astype(NP8)
        chT8[:, kc * K:(kc + 1) * K] = (
            Ch[:, kc * 128:(kc + 1) * 128].T).astype(NP8)
    ch2 = np.zeros((128, SC * D), np.float16)
    cl2 = np.zeros((128, SC * D), np.float16)
    for kc in range(SC):
        ch2[:, kc * D:(kc + 1) * D] = Ch[kc * 128:(kc + 1) * 128]
        cl2[:, kc * D:(kc + 1) * D] = Clf[kc * 128:(kc + 1) * 128]

    R = 128 * T
    in_maps = []
    for core in range(N_CORES):
        bsl = slice(core * BL, (core + 1) * BL)
        ids_c = ids[bsl]
        mask_c = mask[bsl]
        cnt_c = cnt[bsl]

        act_b, act_l = np.nonzero(mask_c)
        na = act_b.shape[0]
        assert na <= R
        ids_packed = np.zeros(R, np.int32)
        ids_packed[:na] = ids_c[act_b, act_l]
        memb = np.full((R,), -1, np.int64)
        memb[:na] = act_b

        ids_tile = np.zeros((128, T), np.int32)
        S = np.zeros((128, T * BL), np.float16)
        for t in range(T):
            rows = np.arange(t * 128, (t + 1) * 128)
            ids_tile[:, t] = ids_packed[rows]
            mb = memb[rows]
            valid = mb >= 0
            S[np.nonzero(valid)[0], t * BL + mb[valid]] = 1.0

        g01 = np.zeros((128, 2 * D), np.float32)
        for t in range(min(2, T)):
            g01[:, t * D:(t + 1) * D] = Emb_scaled[ids_tile[:, t]]
        in_maps.append({
            "emb": Emb_scaled,
            "g01": g01,
            "ids": ids_tile,
            "s16": S,
            "normsb": normsb,
            "cbc": cbc_bcast,
            "chT9": chT9, "cl9T8": cl9T8, "chT8": chT8,
            "ch2": ch2, "cl2": cl2,
            "rvq": (1.0 / float(sc) / cnt_c).astype(np.float32).reshape(BL, 1),
            "rhist": (1.0 / float(sc) / (cnt_c + 1e-9)).astype(np.float32
                                                              ).reshape(BL, 1),
            "corr": (L - cnt_c).astype(np.float32).reshape(BL, 1),
            "wT": wT,
            "bias": bias_bcast,
        })

    nc = _get_program(T, n0)
    res = run_bass_kernel_spmd(nc, in_maps, core_ids=list(range(N_CORES)))
    return np.concatenate([res.results[c]["out"] for c in range(N_CORES)],
                          axis=0)



# revision 17
# speedup vs baseline: 1.8472x; 1.8472x over previous
"""TRN2 Bass kernel for the vq_codebook problem (nn_DNN_34497177321482).

kernel(**inputs) -> np.ndarray  [full-shape in, full-shape out]

Strategy (8 NeuronCores, data-parallel over batch; 64 batches/core):
  - Host packs the active (mask>=1) history positions per core into tiles of
    128 rows and pre-transposes them; the device streams per tile
      xhT [128(dims), 128(rows)]x2 fp16 = fp16(x*16) chunks (main stationary)
      x8T [128, 2, 128] fp8 = fp8(x*16), dims 252-255 replaced by constant
          norm-slot scales (DoubleRow correction stationary)
      gx  [128(rows), 256] fp16 row-major (hist moving)
  - PSUM accumulates sadj = 4096*(||c||^2/2 - x.c) - c0 via:
      main: xhT . chT (fp16, chT = fp16(-c*256)), 8 MMs of free 512
      corr: x8T . cl8T (fp8 DoubleRow, cl8T = fp8((-c*256)-chT) with 4
            contraction slots carrying centered norms at 4 fp8 levels), 4 MMs
  - Act copies each PSUM chunk to SBUF fp16 (scale 1/16; min-region ulp ~4)
  - DVE: folded min (2x pairwise min + reduce), one-hot eq=(sadj16==min) fp8
  - counts: fp8 DoubleRow S^T @ eq over TILE PAIRS into cnt PSUM [64, 2048]
  - hist: S^T @ gx per tile (fp16)
  - stage2: (counts + (L-cnt_b) at n0)^T @ C fp16; [vq,hist]/cnt @ W + b
"""

import sys

sys.path.insert(0, "/opt/trn_rl_repo")

import numpy as np
import ml_dtypes

import concourse.bacc as bacc
import concourse.bass as bass
import concourse.tile as tile
import concourse.mybir as mybir
from concourse.bass_utils import run_bass_kernel_spmd
from concourse.masks import make_identity

F32 = mybir.dt.float32
F16 = mybir.dt.float16
F8 = mybir.dt.float8e4
NP8 = ml_dtypes.float8_e4m3

V, D, K, L, B = 100000, 256, 2048, 200, 512
N_CORES = 8
BL = B // N_CORES
KC = D // 128           # contraction chunks of the distance GEMM
NC = K // 512           # distance n-chunks
SC = K // 128           # stage2 contraction chunks
XC = (2 * D) // 128     # final dense contraction chunks
DD = 4                  # tile DMA lookahead
G2 = 4096.0             # PSUM distance scale
S16 = 1.0 / 16.0        # Act copy scale: PSUM(4096 units) -> fp16
NSLOT = (64.0, 4.0, 0.25, 2.0 ** -6)   # norm-slot stationary scales

_program_cache = {}


def _build_program(T):
    nc = bacc.Bacc("TRN2", target_bir_lowering=False, debug=False,
                   enable_asserts=False, num_devices=N_CORES)
    P = (T + 1) // 2    # tile pairs for fp8 DR counting

    def din(name, shape, dt):
        return nc.dram_tensor(name, shape, dt, kind="ExternalInput").ap()

    xhT_d = din("xhT", [128, T * D], F16)       # per tile: KC blocks of 128
    x8T_d = din("x8T", [128, T * 2, 128], F8)   # per tile: [128, 2, 128]
    gx_d = din("gx", [128, T * D], F16)
    chT_d = din("chT", [128, KC * K], F16)
    cl8T_d = din("cl8T", [128, 2, K], F8)
    s16_d = din("s16", [128, T * BL], F16)
    s8p_d = din("s8p", [128, P * 2, BL], F8)
    c2_d = din("c2", [128, SC * D], F16)
    wT_d = din("wT", [128, XC * D], F16)
    rvq_d = din("rvq", [BL, 1], F32)
    rhist_d = din("rhist", [BL, 1], F32)
    corr_d = din("corr", [BL, 1], F32)
    bias_d = din("bias", [BL, D], F32)
    out_d = nc.dram_tensor("out", [BL, D], F32, kind="ExternalOutput").ap()

    def emit(tc, n0):
        const = tc.alloc_tile_pool(name="const", bufs=1)
        ppersist = tc.alloc_tile_pool(name="ppersist", bufs=1, space="PSUM")

        ident = const.tile([BL, BL], F32, name="ident")
        make_identity(nc, ident[:])

        chT_sb = const.tile([128, KC * K], F16, name="chT_sb")
        for kc in range(KC):
            nc.sync.dma_start(chT_sb[:, kc * K:(kc + 1) * K],
                              chT_d[:, kc * K:(kc + 1) * K])
        cl8T_sb = const.tile([128, 2, K], F8, name="cl8T_sb")
        nc.sync.dma_start(cl8T_sb[:, :, :], cl8T_d)
        s16_sb = const.tile([128, T * BL], F16, name="s16_sb")
        nc.sync.dma_start(s16_sb[:], s16_d)
        s8p_sb = const.tile([128, P * 2, BL], F8, name="s8p_sb")
        nc.sync.dma_start(s8p_sb[:, :, :], s8p_d)
        # late consts (DMAs issued after the tile loop)
        c2_sb = const.tile([128, SC * D], F16, name="c2_sb")
        wT_sb = const.tile([128, XC * D], F16, name="wT_sb")
        rvq_sb = const.tile([BL, 1], F32, name="rvq_sb")
        rhist_sb = const.tile([BL, 1], F32, name="rhist_sb")
        corr_sb = const.tile([BL, 1], F32, name="corr_sb")
        bias_sb = const.tile([BL, D], F32, name="bias_sb")

        cnt_ps = ppersist.tile([BL, K], F32, space="PSUM", name="cnt_ps")
        hist_ps = ppersist.tile([BL, D], F32, space="PSUM", name="hist_ps")

        p_xh = tc.alloc_tile_pool(name="xh", bufs=DD + 2)
        p_x8 = tc.alloc_tile_pool(name="x8", bufs=DD + 2)
        p_gx = tc.alloc_tile_pool(name="gxp", bufs=DD + 2)
        p_sps = tc.alloc_tile_pool(name="sps", bufs=3, space="PSUM")
        p_sadj = tc.alloc_tile_pool(name="sadj", bufs=3)
        p_eq = tc.alloc_tile_pool(name="eq", bufs=2)
        p_m = tc.alloc_tile_pool(name="m", bufs=3)

        def dma_tile(t):
            xh = p_xh.tile([128, D], F16, tag="xh", name=f"xh{t}")
            nc.sync.dma_start(xh[:], xhT_d[:, t * D:(t + 1) * D])
            x8 = p_x8.tile([128, 2, 128], F8, tag="x8", name=f"x8{t}")
            nc.sync.dma_start(x8[:, :, :], x8T_d[:, 2 * t:2 * t + 2, :])
            gx = p_gx.tile([128, D], F16, tag="gx", name=f"gx{t}")
            nc.sync.dma_start(gx[:], gx_d[:, t * D:(t + 1) * D])
            return (xh, x8, gx)

        tiles = {}
        for t in range(min(DD, T)):
            tiles[t] = dma_tile(t)

        eq_tiles = {}

        def emit_cnt(p):
            eqp = eq_tiles.pop(p)
            lhs = s8p_sb[:, 2 * p:2 * p + 2, :]
            for nch in range(NC):
                sl = slice(nch * 512, (nch + 1) * 512)
                nc.tensor.matmul(cnt_ps[:, sl], lhs,
                                 eqp[:, :, sl], start=(p == 0),
                                 stop=(p == P - 1),
                                 perf_mode=mybir.MatmulPerfMode.DoubleRow)

        for t in range(T):
            xh, x8, gx = tiles.pop(t)
            if t + DD < T:
                tiles[t + DD] = dma_tile(t + DD)

            sadj = p_sadj.tile([128, K], F16, tag="sadj", name=f"sadj{t}")
            # chunk pairs: stationary reused across 2 MMs, <=3 chunks alive
            for cp in range(NC // 2):
                s_pair = [p_sps.tile([128, 512], F32, tag="sps", space="PSUM",
                                     name=f"sps{t}_{2 * cp + i}")
                          for i in range(2)]
                for kc in range(KC):
                    for i in range(2):
                        nch = 2 * cp + i
                        csl = slice(kc * K + nch * 512,
                                    kc * K + (nch + 1) * 512)
                        nc.tensor.matmul(s_pair[i][:],
                                         xh[:, kc * 128:(kc + 1) * 128],
                                         chT_sb[:, csl],
                                         start=(kc == 0), stop=False)
                for i in range(2):
                    nch = 2 * cp + i
                    sl = slice(nch * 512, (nch + 1) * 512)
                    nc.tensor.matmul(s_pair[i][:], x8[:, :, :],
                                     cl8T_sb[:, :, sl], start=False, stop=True,
                                     perf_mode=mybir.MatmulPerfMode.DoubleRow)
                    nc.scalar.activation(sadj[:, sl], s_pair[i][:],
                                         mybir.ActivationFunctionType.Copy,
                                         scale=S16)

            # folded min: [2048]->[1024]->[512]->[128,1]
            f1 = p_m.tile([128, 1024], F16, tag="f1", name=f"f1_{t}")
            nc.vector.tensor_tensor(out=f1[:], in0=sadj[:, 0:1024],
                                    in1=sadj[:, 1024:2048],
                                    op=mybir.AluOpType.min)
            f2 = p_m.tile([128, 512], F16, tag="f2", name=f"f2_{t}")
            nc.vector.tensor_tensor(out=f2[:], in0=f1[:, 0:512],
                                    in1=f1[:, 512:1024],
                                    op=mybir.AluOpType.min)
            m32 = p_m.tile([128, 1], F32, tag="m", name=f"m{t}")
            nc.vector.tensor_reduce(out=m32[:], in_=f2[:],
                                    axis=mybir.AxisListType.X,
                                    op=mybir.AluOpType.min)

            pair, slot = t // 2, t % 2
            if slot == 0:
                eqp = p_eq.tile([128, 2, K], F8, tag="eq", name=f"eq{pair}")
                eq_tiles[pair] = eqp
                if t == T - 1:  # odd T: zero the phantom slot once
                    nc.vector.memset(eqp[:, 1, :], 0.0)
            else:
                eqp = eq_tiles[pair]
            nc.vector.tensor_scalar(out=eqp[:, slot, :], in0=sadj[:],
                                    scalar1=m32[:], scalar2=None,
                                    op0=mybir.AluOpType.is_equal)

            nc.tensor.matmul(hist_ps[:], s16_sb[:, t * BL:(t + 1) * BL],
                             gx[:], start=(t == 0), stop=(t == T - 1))

            # counting for a completed pair, two tiles later (keeps PE fed)
            done = (t - 3) // 2 if t >= 3 and (t - 3) % 2 == 0 else None
            if done is not None:
                emit_cnt(done)

        for p in sorted(eq_tiles):
            emit_cnt(p)

        nc.sync.dma_start(c2_sb[:], c2_d)
        nc.sync.dma_start(wT_sb[:], wT_d)
        nc.sync.dma_start(rvq_sb[:], rvq_d)
        nc.sync.dma_start(rhist_sb[:], rhist_d)
        nc.sync.dma_start(corr_sb[:], corr_d)
        nc.sync.dma_start(bias_sb[:], bias_d)

        for p in (p_m, p_eq, p_sadj, p_sps, p_gx, p_x8, p_xh):
            p.release()

        # ---- final phase ----
        fin = tc.alloc_tile_pool(name="fin", bufs=1)
        pfinA = tc.alloc_tile_pool(name="pfinA", bufs=1, space="PSUM")

        cntT = fin.tile([BL, K], F32, name="cntT")
        nc.vector.tensor_copy(cntT[:], cnt_ps[:])
        nc.vector.tensor_tensor(
            out=cntT[:, n0:n0 + 1], in0=cntT[:, n0:n0 + 1],
            in1=corr_sb[:], op=mybir.AluOpType.add)

        vq_ps = pfinA.tile([BL, D], F32, tag="vqps", space="PSUM", name="vqps")
        for kc in range(SC):
            ctp = pfinA.tile([128, BL], F32, tag="ctp", space="PSUM", bufs=2,
                             name=f"ctp{kc}")
            nc.tensor.transpose(ctp[:], cntT[:, kc * 128:(kc + 1) * 128],
                                ident[:])
            cc = fin.tile([128, BL], F16, tag=f"cc{kc % 2}", name=f"cc{kc}")
            nc.vector.tensor_copy(cc[:], ctp[:])
            dsl = slice(kc * D, (kc + 1) * D)
            nc.tensor.matmul(vq_ps[:], cc[:], c2_sb[:, dsl],
                             start=(kc == 0), stop=(kc == SC - 1))

        x_sb = fin.tile([BL, 2 * D], F32, name="x_sb")
        nc.vector.tensor_scalar_mul(x_sb[:, 0:D], vq_ps[:], rvq_sb[:])
        nc.vector.tensor_scalar_mul(x_sb[:, D:2 * D], hist_ps[:], rhist_sb[:])

        pfinA.release()
        pfinB = tc.alloc_tile_pool(name="pfinB", bufs=1, space="PSUM")
        out_ps = pfinB.tile([BL, D], F32, tag="outps", space="PSUM",
                            name="outps")
        for c in range(XC):
            xtp = pfinB.tile([128, BL], F32, tag="xtp", space="PSUM", bufs=2,
                             name=f"xtp{c}")
            nc.tensor.transpose(xtp[:], x_sb[:, c * 128:(c + 1) * 128],
                                ident[:])
            xc = fin.tile([128, BL], F16, tag=f"xc{c % 2}", name=f"xc{c}")
            nc.vector.tensor_copy(xc[:], xtp[:])
            nc.tensor.matmul(out_ps[:], xc[:], wT_sb[:, c * D:(c + 1) * D],
                             start=(c == 0), stop=(c == XC - 1))

        out_sb = fin.tile([BL, D], F32, name="out_sb")
        nc.vector.tensor_tensor(out=out_sb[:], in0=out_ps[:], in1=bias_sb[:],
                                op=mybir.AluOpType.add)
        nc.sync.dma_start(out_d, out_sb[:])

        for p in (pfinB, fin, ppersist, const):
            p.release()

    return nc, emit


def _get_program(T, n0):
    key = (T, n0)
    if key not in _program_cache:
        nc, emit = _build_program(T)
        with tile.TileContext(nc) as tc:
            emit(tc, n0)
        nc.compile()
        _program_cache[key] = nc
    return _program_cache[key]


def kernel(history_item_ids, history_item_masks, embedding_table, code_book,
           W_enc, b_enc):
    ids = np.asarray(history_item_ids)
    masks = np.asarray(history_item_masks)
    E = np.asarray(embedding_table, dtype=np.float32)
    C = np.asarray(code_book, dtype=np.float32)
    W = np.asarray(W_enc, dtype=np.float32)
    b = np.asarray(b_enc, dtype=np.float32)
    assert ids.shape == (B, L) and E.shape == (V, D) and C.shape == (K, D)

    mask = (masks >= 1)
    cnt = mask.sum(axis=1).astype(np.float64)                   # [B]
    n_act = [int(mask[c * BL:(c + 1) * BL].sum()) for c in range(N_CORES)]
    T = max(DD, -(-max(n_act) // 128))
    P = (T + 1) // 2

    norms64 = (C.astype(np.float64) ** 2).sum(axis=1)
    norms32 = (C ** 2).sum(axis=1, dtype=np.float32)
    n0 = int(np.argmin(norms32))

    E16 = E.astype(np.float16)
    cn = C.astype(np.float64) * -256.0
    chT16 = cn.astype(np.float16)                               # fp16(-c*256)
    clr8 = (cn - chT16.astype(np.float64)).astype(np.float32).astype(NP8)
    C16 = C.astype(np.float16)

    # center c0: median row-min over a sample of active rows
    act_b0, act_l0 = np.nonzero(mask[:2])
    samp = E16[ids[:2][act_b0[:512], act_l0[:512]]].astype(np.float32)
    normsT = norms64 * G2 / 2.0
    sadj_s = normsT[None, :].astype(np.float32) + \
        (samp * 16.0) @ chT16.astype(np.float32).T
    c0 = float(np.median(sadj_s.min(axis=1)))

    # norm levels -> fp8 on 4 donated contraction slots (dims 252..255)
    res = (normsT - c0).copy()
    nlev = []
    for s in NSLOT:
        q = (res / s).astype(np.float32).astype(NP8)
        nlev.append(q)
        res -= q.astype(np.float64) * s

    # cl8T [p, j, n] = residual of c-dim j*128+p for code n
    cl8T = np.zeros((128, 2, K), NP8)
    for j in range(2):
        cl8T[:, j, :] = clr8[:, j * 128:(j + 1) * 128].T
    for i in range(len(NSLOT)):
        dslot = 252 + i
        cl8T[dslot % 128, dslot // 128, :] = nlev[i]

    c2 = np.zeros((128, SC * D), np.float16)
    for kc in range(SC):
        c2[:, kc * D:(kc + 1) * D] = C16[kc * 128:(kc + 1) * 128]
    wT = np.zeros((128, XC * D), np.float16)
    for c in range(XC):
        wT[:, c * D:(c + 1) * D] = W[c * 128:(c + 1) * 128].astype(np.float16)
    bias_bcast = np.broadcast_to(b, (BL, D)).copy()
    chT = np.zeros((128, KC * K), np.float16)
    for kc in range(KC):
        chT[:, kc * K:(kc + 1) * K] = chT16[:, kc * 128:(kc + 1) * 128].T
    slot_scales = np.array(NSLOT, np.float32).astype(NP8)

    R = 128 * T
    in_maps = []
    for core in range(N_CORES):
        bsl = slice(core * BL, (core + 1) * BL)
        ids_c = ids[bsl]
        mask_c = mask[bsl]
        cnt_c = cnt[bsl]

        act_b, act_l = np.nonzero(mask_c)
        na = act_b.shape[0]
        assert na <= R
        ids_packed = np.zeros(R, np.int32)
        ids_packed[:na] = ids_c[act_b, act_l]
        memb = np.full((R,), -1, np.int64)
        memb[:na] = act_b

        rows16 = E16[ids_packed].copy()             # [R, 256] fp16
        rows16[na:] = 0
        x16 = (rows16.astype(np.float32) * 16.0).astype(np.float16)
        x8 = x16.astype(NP8)

        xhT = np.zeros((128, T * D), np.float16)
        x8T = np.zeros((128, T * 2, 128), NP8)
        gx16 = np.zeros((128, T * D), np.float16)
        S16m = np.zeros((128, T * BL), np.float16)
        s8p = np.zeros((128, P * 2, BL), NP8)
        for t in range(T):
            rows = slice(t * 128, (t + 1) * 128)
            blk = x16[rows]                         # [128 rows, 256]
            for kc in range(KC):
                xhT[:, t * D + kc * 128: t * D + (kc + 1) * 128] = \
                    blk[:, kc * 128:(kc + 1) * 128].T
            b8 = x8[rows]
            x8T[:, 2 * t + 0, :] = b8[:, 0:128].T
            x8T[:, 2 * t + 1, :] = b8[:, 128:256].T
            for i in range(len(NSLOT)):
                dslot = 252 + i
                x8T[dslot % 128, 2 * t + dslot // 128, :] = slot_scales[i]
            gx16[:, t * D:(t + 1) * D] = rows16[rows]
            mb = memb[rows]
            valid = np.nonzero(mb >= 0)[0]
            S16m[valid, t * BL + mb[valid]] = 1.0
            s8p[valid, t, mb[valid]] = 1.0

        in_maps.append({
            "xhT": xhT,
            "x8T": x8T,
            "gx": gx16,
            "chT": chT,
            "cl8T": cl8T,
            "s16": S16m,
            "s8p": s8p,
            "c2": c2,
            "wT": wT,
            "rvq": (1.0 / cnt_c).astype(np.float32).reshape(BL, 1),
            "rhist": (1.0 / (cnt_c + 1e-9)).astype(np.float32).reshape(BL, 1),
            "corr": (L - cnt_c).astype(np.float32).reshape(BL, 1),
            "bias": bias_bcast,
        })

    nc = _get_program(T, n0)
    res = run_bass_kernel_spmd(nc, in_maps, core_ids=list(range(N_CORES)))
    return np.concatenate([res.results[c]["out"] for c in range(N_CORES)],
                          axis=0)
